# revision 10
# baseline (speedup 1.0000x reference)
"""Trainium2 Bass kernel for nn_BasicBlockA (masked-conv flow block).

Computation: two 3x3 convs with tiny channel counts (3->12, 12->3),
leaky-relu between, residual add, plus a log-det reduction.

Mapping: "banded stationary" matmuls.  A block of 8 output rows is computed
per step; the PE contraction dim holds (channel, band-row) pairs and the
3x3 conv's dy-taps live inside a banded stationary matrix, while the dx-taps
become 3 PSUM-accumulating matmuls whose moving operands are x-shifted views
of the same SBUF tile.  conv1 emits lat1 in exactly the (channel, band-row)
partition layout conv2 consumes, so no on-chip relayout is needed.  The
residual (res*x) is folded in as a 4th accumulating matmul (full fp32 from
its own x tile), and the log-det diagonal sum is one more matmul over the
sign bits with the masks/centers baked into the stationary host-side.

Data parallel over batch: 4 images per core x 8 cores; images processed in
pairs side-by-side in the free dim (2 x 258 incl. zero-pad columns -> N=512).
"""
import os
import sys

sys.path.insert(0, "/opt/trn_rl_repo")

import numpy as np

L, C, KK, MID = 4, 3, 3, 1
SLOPE = 0.1
H = W = 256
N_CORES = 8
NI = 4            # images per core
NP = 2            # images per pair (batched in matmul free dim)
NB = H // 8       # 8-output-row blocks per pair
FP = W + 2        # padded row length

CONV_DTYPE = os.environ.get("BASS_CONV_DTYPE", "bf16")   # "bf16" | "fp32"

_cache = {}


def _masks():
    m0 = np.ones((C, C, KK, KK), np.float32)
    m1 = np.zeros((C, C, KK, KK), np.float32)
    m = np.ones((C, C, KK, KK), np.float32)
    for i in range(C):
        m0[i, i, MID, MID] = 0.0
        m1[i, i, MID, MID] = 1.0
        m[i, :, MID + 1:, :] = 0.0
        m[i, :i + 1, MID, MID + 1:] = 0.0
        m[i, i + 1:, MID, MID:] = 0.0
    return m0, m1, m


def _prep_consts(w1, b1, centers, w2, b2, res):
    """Host-side: fold masks/weights into banded stationaries + bias vectors."""
    m0, m1, m = _masks()
    W1f = ((w1 * m0 + centers * m1) * m).reshape(L * C, C, KK, KK)
    w2r = w2.reshape(L, L, C, C, KK, KK)
    Wsum = (w2r.sum(0) * m0 + L * (centers * m1)) * m
    Wcat = Wsum.transpose(1, 0, 2, 3, 4).reshape(C, L * C, KK, KK)
    bsum = b2.sum(0)
    cdsq = centers[:, np.arange(C), np.arange(C), MID, MID] ** 2   # [L, C]
    S_c = cdsq.sum(0)
    rm = float(res[0] * (res[0] > 0))
    b1f = b1.reshape(L * C)

    # conv1 stationaries [variant(0=mid,1=top,2=bot)][dx][36=(yl,ci)][120=(co,yo)]
    w1s = np.zeros((3, 3, 36, 120), np.float32)
    yl = np.arange(12)[:, None]
    yo = np.arange(10)[None, :]
    band = (yl - yo >= 0) & (yl - yo <= 2)
    for var in range(3):
        for dx in range(3):
            for ci in range(C):
                for co in range(L * C):
                    blk = np.where(band, W1f[co, ci, np.clip(yl - yo, 0, 2), dx], 0.0)
                    if var == 1:
                        blk = blk.copy(); blk[:, 0] = 0.0
                    if var == 2:
                        blk = blk.copy(); blk[:, 9] = 0.0
                    w1s[var, dx, ci::3, co * 10:co * 10 + 10] = blk
    # conv2 stationaries [dx][120=(ci2,yl2)][24=(c,yo2)]
    w2s = np.zeros((3, 120, 24), np.float32)
    yl2 = np.arange(10)[:, None]
    yo2 = np.arange(8)[None, :]
    band2 = (yl2 - yo2 >= 0) & (yl2 - yo2 <= 2)
    for dx in range(3):
        for ci2 in range(L * C):
            for c in range(C):
                blk = np.where(band2, Wcat[c, ci2, np.clip(yl2 - yo2, 0, 2), dx], 0.0)
                w2s[dx, ci2 * 10:ci2 * 10 + 10, c * 8:c * 8 + 8] = blk
    # residual stationary [24=(ylr,ci)][24=(c,yo2)] : 16*rm at ylr==yo2, ci==c
    wxs = np.zeros((24, 24), np.float32)
    for ylr in range(8):
        for c in range(C):
            wxs[ylr * 3 + c, c * 8 + ylr] = 16.0 * rm
    # log-det sum stationary [120=((l,c),yo)][32=(c,yo)+2pad]
    wls = np.zeros((120, 32), np.float32)
    for l in range(L):
        for c in range(C):
            for yob in range(1, 9):
                wls[(l * 3 + c) * 10 + yob, c * 10 + yob] = 0.9 * cdsq[l, c]
    # conv1 bias variants [3][120]
    b1v = np.tile(np.repeat(b1f, 10).astype(np.float32), (3, 1))
    b1v[1, 0::10] = 0.0
    b1v[2, 9::10] = 0.0
    # log bias [128]: rm + 0.1*S_c interior, 1.0 elsewhere (-> ln 0)
    bl = np.ones(128, np.float32)
    for g in range(4):
        for c in range(C):
            bl[32 * g + c * 10 + 1:32 * g + c * 10 + 9] = rm + 0.1 * S_c[c]
    b2v = np.repeat(bsum / 16.0, 8).astype(np.float32)
    # device tiles are partition-major: [36,(var,dx),120] and [120,(dx),24]
    w1s = np.ascontiguousarray(w1s.transpose(2, 0, 1, 3))
    w2s = np.ascontiguousarray(w2s.transpose(1, 0, 2))
    if CONV_DTYPE == "bf16":
        import ml_dtypes
        w1s = w1s.astype(ml_dtypes.bfloat16)
        w2s = w2s.astype(ml_dtypes.bfloat16)
    return {
        "w1s": w1s, "w2s": w2s, "wxs": wxs, "wls": wls,
        "b1s": b1v, "bls": bl, "b2s": b2v,
    }


def _build_nc():
    import concourse.bacc as bacc
    import concourse.bass as bass
    import concourse.tile as tile
    from concourse import mybir

    f32 = mybir.dt.float32
    ct = mybir.dt.bfloat16 if CONV_DTYPE == "bf16" else f32
    AF = mybir.ActivationFunctionType
    ALU = mybir.AluOpType
    AP = bass.AP

    nc = bacc.Bacc("TRN2", target_bir_lowering=False, debug=False,
                   num_devices=N_CORES)
    xs = nc.dram_tensor("xs", [NI, C, H, W], f32, kind="ExternalInput")
    w1s = nc.dram_tensor("w1s", [36, 3, 3, 120], ct, kind="ExternalInput")
    w2s = nc.dram_tensor("w2s", [120, 3, 24], ct, kind="ExternalInput")
    wxs = nc.dram_tensor("wxs", [24, 24], f32, kind="ExternalInput")
    wls = nc.dram_tensor("wls", [120, 32], f32, kind="ExternalInput")
    b1s = nc.dram_tensor("b1s", [3, 120], f32, kind="ExternalInput")
    bls = nc.dram_tensor("bls", [128], f32, kind="ExternalInput")
    b2s = nc.dram_tensor("b2s", [24], f32, kind="ExternalInput")
    ys = nc.dram_tensor("ys", [NI, C, H, W], f32, kind="ExternalOutput")
    lds = nc.dram_tensor("lds", [128], f32, kind="ExternalOutput")

    with tile.TileContext(nc) as tc:
        with (
            tc.tile_pool(name="singles", bufs=1) as singles,
            tc.tile_pool(name="xfp", bufs=4) as xfp,
            tc.tile_pool(name="dsp", bufs=3) as dsp,
            tc.tile_pool(name="outp", bufs=4) as outp,
            tc.tile_pool(name="lgp", bufs=2) as lgp,
            tc.tile_pool(name="ps1p", bufs=2, space="PSUM") as ps1p,
            tc.tile_pool(name="ps2p", bufs=3, space="PSUM") as ps2p,
            tc.tile_pool(name="ps3p", bufs=2, space="PSUM") as ps3p,
        ):
            w1_sb = singles.tile([36, 3, 3, 120], ct)
            nc.sync.dma_start(out=w1_sb[:], in_=w1s.ap())
            w2_sb = singles.tile([120, 3, 24], ct)
            nc.sync.dma_start(out=w2_sb[:], in_=w2s.ap())
            wx_sb = singles.tile([24, 24], f32)
            nc.sync.dma_start(out=wx_sb[:], in_=wxs.ap())
            wl_sb = singles.tile([120, 32], f32)
            nc.sync.dma_start(out=wl_sb[:], in_=wls.ap())
            b1_sb = singles.tile([120, 3], f32)
            for var in range(3):
                nc.sync.dma_start(out=b1_sb[:, var:var + 1],
                                  in_=b1s.ap()[var:var + 1, :].transpose([1, 0]))
            bl_sb = singles.tile([128, 1], f32)
            nc.sync.dma_start(out=bl_sb[:], in_=bls.ap().unsqueeze(1))
            b2_sb = singles.tile([24, 1], f32)
            nc.sync.dma_start(out=b2_sb[:], in_=b2s.ap().unsqueeze(1))
            acc = singles.tile([128, NP, W], f32)
            nc.vector.memset(acc[:], 0.0)
            ld_sb = singles.tile([128, 1], f32)
            # persistent rotated buffers for the zero-padded tiles: pad columns
            # are zeroed once and never touched again
            xt_bufs = [singles.tile([36, NP, FP], ct, name=f"xtb{i}",
                                    tag=f"xtb{i}") for i in range(4)]
            lat_bufs = [singles.tile([120, NP, FP], ct, name=f"latb{i}",
                                     tag=f"latb{i}") for i in range(3)]
            for t in xt_bufs + lat_bufs:
                nc.vector.memset(t[:, :, 0:1], 0.0)
                nc.vector.memset(t[:, :, FP - 1:FP], 0.0)

            for pair in range(NI // NP):
                ps3 = None
                for b in range(NB):
                    tglob = pair * NB + b
                    var = 1 if b == 0 else (2 if b == NB - 1 else 0)
                    # ---- x band (conv dtype): partitions yl*3+ci, rows 8b-2..8b+9
                    xt = xt_bufs[tglob % 4]
                    xta = xt[:]
                    XPI = xta.ap[0][0]
                    yl0, nyl = 0, 12
                    if b == 0:
                        yl0, nyl = 2, 10
                        nc.vector.memset(xt[0:6, :, 1:W + 1], 0.0)
                    if b == NB - 1:
                        nyl = 10
                        nc.vector.memset(xt[:, :, 1:W + 1], 0.0)
                    r0 = 8 * b - 2 + yl0
                    for ci in range(C):
                        dst = AP(tensor=xta.tensor,
                                 offset=xta.offset + (yl0 * 3 + ci) * XPI + 1,
                                 ap=[[3 * XPI, nyl], [FP, NP], [1, W]])
                        src = AP(tensor=xs.ap().tensor,
                                 offset=(pair * NP) * C * H * W + ci * H * W + r0 * W,
                                 ap=[[W, nyl], [C * H * W, NP], [1, W]])
                        nc.gpsimd.dma_start(out=dst, in_=src)
                    # ---- x rows 8b..8b+7 (fp32) for the residual matmul
                    xf = xfp.tile([24, NP, W], f32)
                    xfa = xf[:]
                    XFI = xfa.ap[0][0]
                    for ci in range(C):
                        dst = AP(tensor=xfa.tensor, offset=xfa.offset + ci * XFI,
                                 ap=[[3 * XFI, 8], [FP - 2, NP], [1, W]])
                        src = AP(tensor=xs.ap().tensor,
                                 offset=(pair * NP) * C * H * W + ci * H * W + 8 * b * W,
                                 ap=[[W, 8], [C * H * W, NP], [1, W]])
                        nc.sync.dma_start(out=dst, in_=src)
                    # ---- conv1: 3 dx-phases into psum1 [120=(co,yo), NP, W]
                    ps1 = ps1p.tile([120, NP, W], f32)
                    for dx in range(3):
                        rhs = AP(tensor=xta.tensor, offset=xta.offset + dx,
                                 ap=[[XPI, 36], [FP, NP], [1, W]])
                        nc.tensor.matmul(ps1[:], w1_sb[:, var, dx, :], rhs,
                                         start=(dx == 0), stop=(dx == 2))
                    # ---- lat1 = lrelu(psum1 + b1)  (conv dtype out)
                    lat = lat_bufs[tglob % 3]
                    nc.scalar.activation(out=lat[:, :, 1:W + 1], in_=ps1[:],
                                         func=AF.Prelu, bias=b1_sb[:, var:var + 1],
                                         scale=1.0, alpha=SLOPE)
                    # ---- ds = (lat1 > 0) for the jacobian diagonal
                    ds = dsp.tile([120, NP, W], f32)
                    nc.vector.tensor_scalar(out=ds[:], in0=lat[:, :, 1:W + 1],
                                            scalar1=0.0, scalar2=None, op0=ALU.is_gt)
                    # ---- conv2 (3 dx-phases) + residual matmul into psum2
                    ps2 = ps2p.tile([24, NP, W], f32)
                    lata = lat[:]
                    LPI = lata.ap[0][0]
                    for dx in range(3):
                        rhs = AP(tensor=lata.tensor, offset=lata.offset + dx,
                                 ap=[[LPI, 120], [FP, NP], [1, W]])
                        nc.tensor.matmul(ps2[:], w2_sb[:, dx, :], rhs,
                                         start=(dx == 0), stop=False)
                    nc.tensor.matmul(ps2[:], wx_sb[:], xf[:], start=False, stop=True)
                    # ---- out = psum2/16 + bsum/16 ; store
                    osb = outp.tile([24, NP, W], f32)
                    nc.scalar.activation(out=osb[:], in_=ps2[:], func=AF.Identity,
                                         bias=b2_sb[:, 0:1], scale=1.0 / 16.0)
                    for c in range(C):
                        dst = AP(tensor=ys.ap().tensor,
                                 offset=(pair * NP) * C * H * W + c * H * W + (8 * b) * W,
                                 ap=[[W, 8], [C * H * W, NP], [1, W]])
                        nc.sync.dma_start(out=dst, in_=osb[c * 8:(c + 1) * 8])
                    # ---- log-det: diag partial sums into psum3 column group b%4
                    g = b % 4
                    if g == 0:
                        ps3 = ps3p.tile([128, NP, W], f32)
                    nc.tensor.matmul(ps3[32 * g:32 * g + 32], wl_sb[:], ds[:],
                                     start=True, stop=True, tile_position=(0, 32 * g))
                    if g == 3:
                        lg = lgp.tile([128, NP, W], f32)
                        nc.scalar.activation(out=lg[:], in_=ps3[:], func=AF.Ln,
                                             bias=bl_sb[:, 0:1], scale=1.0)
                        nc.vector.scalar_tensor_tensor(out=acc[:], in0=lg[:],
                                                       scalar=0.0, in1=acc[:],
                                                       op0=ALU.add, op1=ALU.add)
            # ---- final: per-partition sum of acc -> lds
            lgf = lgp.tile([128, NP, W], f32)
            nc.scalar.activation(out=lgf[:], in_=acc[:], func=AF.Identity,
                                 bias=0.0, scale=1.0, accum_out=ld_sb[:])
            nc.sync.dma_start(out=lds.ap().unsqueeze(1), in_=ld_sb[:])

    nc.compile()
    return nc


def kernel(x, log_det, w1, b1, centers, w2, b2, res):
    x = np.ascontiguousarray(np.asarray(x, np.float32))
    log_det = np.asarray(log_det, np.float32)
    consts = _prep_consts(np.asarray(w1, np.float32), np.asarray(b1, np.float32),
                          np.asarray(centers, np.float32), np.asarray(w2, np.float32),
                          np.asarray(b2, np.float32), np.asarray(res, np.float32))

    if "nc" not in _cache:
        _cache["nc"] = _build_nc()
    nc = _cache["nc"]

    from concourse.bass_utils import run_bass_kernel_spmd
    in_maps = []
    for cid in range(N_CORES):
        m = dict(consts)
        m["xs"] = x[cid * NI:(cid + 1) * NI]
        in_maps.append(m)
    res_ = run_bass_kernel_spmd(nc, in_maps, core_ids=list(range(N_CORES)))

    out = np.concatenate([r["ys"] for r in res_.results], axis=0)
    ld_total = np.sum([np.sum(r["lds"], dtype=np.float64) for r in res_.results])
    log_det_out = (log_det.astype(np.float64) + ld_total).astype(np.float32)
    log_det_out = np.asarray(log_det_out, np.float32).reshape(np.shape(log_det))
    return out, log_det_out


# revision 13
# speedup vs baseline: 1.7273x; 1.7273x over previous
"""Trainium2 Bass kernel for nn_BasicBlockA (masked-conv flow block).

Computation: two 3x3 convs with tiny channel counts (3->12, 12->3),
leaky-relu between, residual add, plus a log-det reduction.

Mapping: "banded stationary" matmuls.  A block of 8 output rows is computed
per step; the PE contraction dim holds (channel, band-row) pairs and the
3x3 conv's dy-taps live inside a banded stationary matrix, while the dx-taps
become 3 PSUM-accumulating matmuls whose moving operands are x-shifted views
of the same SBUF tile.  conv1 emits lat1 in exactly the (channel, band-row)
partition layout conv2 consumes, so no on-chip relayout is needed.  The
residual (res*x) is folded in as a 4th accumulating matmul (full fp32 from
its own x tile), and the log-det diagonal sum is one more matmul over the
sign bits with the masks/centers baked into the stationary host-side.

Data parallel over batch: 4 images per core x 8 cores; images processed in
pairs side-by-side in the free dim (2 x 258 incl. zero-pad columns -> N=512).
"""
import os
import sys

sys.path.insert(0, "/opt/trn_rl_repo")

import numpy as np

L, C, KK, MID = 4, 3, 3, 1
SLOPE = 0.1
H = W = 256
N_CORES = 8
NI = 4            # images per core
NP = 2            # images per pair (batched in matmul free dim)
NB = H // 8       # 8-output-row blocks per pair
FP = W + 2        # padded row length

CONV_DTYPE = os.environ.get("BASS_CONV_DTYPE", "bf16")   # "bf16" | "fp32"

_cache = {}


def _masks():
    m0 = np.ones((C, C, KK, KK), np.float32)
    m1 = np.zeros((C, C, KK, KK), np.float32)
    m = np.ones((C, C, KK, KK), np.float32)
    for i in range(C):
        m0[i, i, MID, MID] = 0.0
        m1[i, i, MID, MID] = 1.0
        m[i, :, MID + 1:, :] = 0.0
        m[i, :i + 1, MID, MID + 1:] = 0.0
        m[i, i + 1:, MID, MID:] = 0.0
    return m0, m1, m


def _prep_consts(w1, b1, centers, w2, b2, res):
    """Host-side: fold masks/weights into banded stationaries + bias vectors."""
    m0, m1, m = _masks()
    W1f = ((w1 * m0 + centers * m1) * m).reshape(L * C, C, KK, KK)
    w2r = w2.reshape(L, L, C, C, KK, KK)
    Wsum = (w2r.sum(0) * m0 + L * (centers * m1)) * m
    Wcat = Wsum.transpose(1, 0, 2, 3, 4).reshape(C, L * C, KK, KK)
    bsum = b2.sum(0)
    cdsq = centers[:, np.arange(C), np.arange(C), MID, MID] ** 2   # [L, C]
    S_c = cdsq.sum(0)
    rm = float(res[0] * (res[0] > 0))
    b1f = b1.reshape(L * C)

    # conv1 stationaries [variant(0=mid,1=top,2=bot)][dx][36=(yl,ci)][120=(co,yo)]
    w1s = np.zeros((3, 3, 36, 120), np.float32)
    yl = np.arange(12)[:, None]
    yo = np.arange(10)[None, :]
    band = (yl - yo >= 0) & (yl - yo <= 2)
    for var in range(3):
        for dx in range(3):
            for ci in range(C):
                for co in range(L * C):
                    blk = np.where(band, W1f[co, ci, np.clip(yl - yo, 0, 2), dx], 0.0)
                    if var == 1:
                        blk = blk.copy(); blk[:, 0] = 0.0
                    if var == 2:
                        blk = blk.copy(); blk[:, 9] = 0.0
                    w1s[var, dx, ci::3, co * 10:co * 10 + 10] = blk
    # conv2 stationaries [dx][120=(ci2,yl2)][24=(c,yo2)]
    w2s = np.zeros((3, 120, 24), np.float32)
    yl2 = np.arange(10)[:, None]
    yo2 = np.arange(8)[None, :]
    band2 = (yl2 - yo2 >= 0) & (yl2 - yo2 <= 2)
    for dx in range(3):
        for ci2 in range(L * C):
            for c in range(C):
                blk = np.where(band2, Wcat[c, ci2, np.clip(yl2 - yo2, 0, 2), dx], 0.0)
                w2s[dx, ci2 * 10:ci2 * 10 + 10, c * 8:c * 8 + 8] = blk
    rmv = np.full(24, rm, np.float32)
    # log-det sum stationary [120=((l,c),yo)][32=(c,yo)+2pad]
    wls = np.zeros((120, 32), np.float32)
    for l in range(L):
        for c in range(C):
            for yob in range(1, 9):
                wls[(l * 3 + c) * 10 + yob, c * 10 + yob] = 0.9 * cdsq[l, c]
    # conv1 bias variants [3][120]
    b1v = np.tile(np.repeat(b1f, 10).astype(np.float32), (3, 1))
    b1v[1, 0::10] = 0.0
    b1v[2, 9::10] = 0.0
    # log bias [128]: rm + 0.1*S_c interior, 1.0 elsewhere (-> ln 0)
    bl = np.ones(128, np.float32)
    for g in range(4):
        for c in range(C):
            bl[32 * g + c * 10 + 1:32 * g + c * 10 + 9] = rm + 0.1 * S_c[c]
    b2v = np.repeat(bsum / 16.0, 8).astype(np.float32)
    # device tiles are partition-major: [36,(var,dx),120] and [120,(dx),24]
    w1s = np.ascontiguousarray(w1s.transpose(2, 0, 1, 3))
    w2s = np.ascontiguousarray(w2s.transpose(1, 0, 2))
    if CONV_DTYPE == "bf16":
        import ml_dtypes
        w1s = w1s.astype(ml_dtypes.bfloat16)
        w2s = w2s.astype(ml_dtypes.bfloat16)
    return {
        "w1s": w1s, "w2s": w2s, "rmv": rmv, "wls": wls,
        "b1s": b1v, "bls": bl, "b2s": b2v,
    }


def _build_nc():
    import concourse.bacc as bacc
    import concourse.bass as bass
    import concourse.tile as tile
    from concourse import mybir

    f32 = mybir.dt.float32
    ct = mybir.dt.bfloat16 if CONV_DTYPE == "bf16" else f32
    AF = mybir.ActivationFunctionType
    ALU = mybir.AluOpType
    AP = bass.AP

    nc = bacc.Bacc("TRN2", target_bir_lowering=False, debug=False,
                   num_devices=N_CORES)
    xs = nc.dram_tensor("xs", [NI, C, H, W], f32, kind="ExternalInput")
    w1s = nc.dram_tensor("w1s", [36, 3, 3, 120], ct, kind="ExternalInput")
    w2s = nc.dram_tensor("w2s", [120, 3, 24], ct, kind="ExternalInput")
    rmv = nc.dram_tensor("rmv", [24], f32, kind="ExternalInput")
    wls = nc.dram_tensor("wls", [120, 32], f32, kind="ExternalInput")
    b1s = nc.dram_tensor("b1s", [3, 120], f32, kind="ExternalInput")
    bls = nc.dram_tensor("bls", [128], f32, kind="ExternalInput")
    b2s = nc.dram_tensor("b2s", [24], f32, kind="ExternalInput")
    ys = nc.dram_tensor("ys", [NI, C, H, W], f32, kind="ExternalOutput")
    lds = nc.dram_tensor("lds", [128], f32, kind="ExternalOutput")

    with tile.TileContext(nc) as tc:
        with (
            tc.tile_pool(name="singles", bufs=1) as singles,
            tc.tile_pool(name="xfp", bufs=1) as xfp,
            tc.tile_pool(name="dsp", bufs=3) as dsp,
            tc.tile_pool(name="outp", bufs=1) as outp,
            tc.tile_pool(name="lgp", bufs=2) as lgp,
            tc.tile_pool(name="ps1p", bufs=2, space="PSUM") as ps1p,
            tc.tile_pool(name="ps2p", bufs=3, space="PSUM") as ps2p,
            tc.tile_pool(name="ps3p", bufs=2, space="PSUM") as ps3p,
        ):
            w1_sb = singles.tile([36, 3, 3, 120], ct)
            nc.sync.dma_start(out=w1_sb[:], in_=w1s.ap())
            w2_sb = singles.tile([120, 3, 24], ct)
            nc.sync.dma_start(out=w2_sb[:], in_=w2s.ap())
            rm_sb = singles.tile([24, 1], f32)
            nc.sync.dma_start(out=rm_sb[:], in_=rmv.ap().unsqueeze(1))
            wl_sb = singles.tile([120, 32], f32)
            nc.sync.dma_start(out=wl_sb[:], in_=wls.ap())
            b1_sb = singles.tile([120, 3], f32)
            for var in range(3):
                nc.sync.dma_start(out=b1_sb[:, var:var + 1],
                                  in_=b1s.ap()[var:var + 1, :].transpose([1, 0]))
            bl_sb = singles.tile([128, 1], f32)
            nc.sync.dma_start(out=bl_sb[:], in_=bls.ap().unsqueeze(1))
            b2_sb = singles.tile([24, 1], f32)
            nc.sync.dma_start(out=b2_sb[:], in_=b2s.ap().unsqueeze(1))
            acc = singles.tile([128, NP, W], f32)
            nc.vector.memset(acc[:], 0.0)
            ld_sb = singles.tile([128, 1], f32)
            # persistent rotated buffers for the zero-padded tiles: pad columns
            # are zeroed once and never touched again
            xt_bufs = [singles.tile([36, NP, FP], ct, name=f"xtb{i}",
                                    tag=f"xtb{i}") for i in range(4)]
            lat_bufs = [singles.tile([120, NP, FP], ct, name=f"latb{i}",
                                     tag=f"latb{i}") for i in range(3)]
            for t in xt_bufs + lat_bufs:
                nc.vector.memset(t[:, :, 0:1], 0.0)
                nc.vector.memset(t[:, :, FP - 1:FP], 0.0)

            for pair in range(NI // NP):
                # residual input for the whole pair: [24=(c,yo2), img, blk, x]
                xf_all = xfp.tile([24, NP, NB, W], f32)
                xfa = xf_all[:]
                XFI = xfa.ap[0][0]
                for img in range(NP):
                    for c in range(C):
                        dst = AP(tensor=xfa.tensor,
                                 offset=xfa.offset + (c * 8) * XFI + img * NB * W,
                                 ap=[[XFI, 8], [W, NB], [1, W]])
                        src = AP(tensor=xs.ap().tensor,
                                 offset=(pair * NP + img) * C * H * W + c * H * W,
                                 ap=[[W, 8], [8 * W, NB], [1, W]])
                        nc.sync.dma_start(out=dst, in_=src)
                nc.vector.tensor_scalar(out=xf_all[:], in0=xf_all[:],
                                        scalar1=rm_sb[:, 0:1], scalar2=b2_sb[:, 0:1],
                                        op0=ALU.mult, op1=ALU.add)
                out_all = outp.tile([24, NP, NB, W], f32)
                oaa = out_all[:]
                OAI = oaa.ap[0][0]
                ps3 = None
                for b in range(NB):
                    tglob = pair * NB + b
                    var = 1 if b == 0 else (2 if b == NB - 1 else 0)
                    # ---- x band (conv dtype): partitions yl*3+ci, rows 8b-2..8b+9
                    xt = xt_bufs[tglob % 4]
                    xta = xt[:]
                    XPI = xta.ap[0][0]
                    yl0, nyl = 0, 12
                    if b == 0:
                        yl0, nyl = 2, 10
                        nc.vector.memset(xt[0:6, :, 1:W + 1], 0.0)
                    if b == NB - 1:
                        nyl = 10
                        nc.vector.memset(xt[:, :, 1:W + 1], 0.0)
                    r0 = 8 * b - 2 + yl0
                    for img in range(NP):
                        dst = AP(tensor=xta.tensor,
                                 offset=xta.offset + (yl0 * 3) * XPI + img * FP + 1,
                                 ap=[[XPI, nyl * 3], [1, W]])
                        src = AP(tensor=xs.ap().tensor,
                                 offset=(pair * NP + img) * C * H * W + r0 * W,
                                 ap=[[W, nyl], [H * W, C], [1, W]])
                        nc.gpsimd.dma_start(out=dst, in_=src)
                    # ---- conv1: 3 dx-phases into psum1 [120=(co,yo), NP, W]
                    ps1 = ps1p.tile([120, NP, W], f32)
                    for dx in range(3):
                        rhs = AP(tensor=xta.tensor, offset=xta.offset + dx,
                                 ap=[[XPI, 36], [FP, NP], [1, W]])
                        nc.tensor.matmul(ps1[:], w1_sb[:, var, dx, :], rhs,
                                         start=(dx == 0), stop=(dx == 2))
                    # ---- lat1 = lrelu(psum1 + b1)  (conv dtype out)
                    lat = lat_bufs[tglob % 3]
                    nc.scalar.activation(out=lat[:, :, 1:W + 1], in_=ps1[:],
                                         func=AF.Prelu, bias=b1_sb[:, var:var + 1],
                                         scale=1.0, alpha=SLOPE)
                    # ---- ds = (lat1 > 0) for the jacobian diagonal
                    ds = dsp.tile([120, NP, W], f32)
                    nc.vector.tensor_scalar(out=ds[:], in0=lat[:, :, 1:W + 1],
                                            scalar1=0.0, scalar2=None, op0=ALU.is_gt)
                    # ---- conv2 (3 dx-phases) + residual matmul into psum2
                    ps2 = ps2p.tile([24, NP, W], f32)
                    lata = lat[:]
                    LPI = lata.ap[0][0]
                    for dx in range(3):
                        rhs = AP(tensor=lata.tensor, offset=lata.offset + dx,
                                 ap=[[LPI, 120], [FP, NP], [1, W]])
                        nc.tensor.matmul(ps2[:], w2_sb[:, dx, :], rhs,
                                         start=(dx == 0), stop=(dx == 2))
                    # ---- out = psum2/16 + (bsum/16 + rm*x)  -> out_all[:, :, b, :]
                    xfs_sl = AP(tensor=xfa.tensor, offset=xfa.offset + b * W,
                                ap=[[XFI, 24], [NB * W, NP], [1, W]])
                    out_sl = AP(tensor=oaa.tensor, offset=oaa.offset + b * W,
                                ap=[[OAI, 24], [NB * W, NP], [1, W]])
                    nc.vector.scalar_tensor_tensor(out=out_sl, in0=ps2[:],
                                                   scalar=1.0 / 16.0, in1=xfs_sl,
                                                   op0=ALU.mult, op1=ALU.add)
                    # ---- log-det: diag partial sums into psum3 column group b%4
                    g = b % 4
                    if g == 0:
                        ps3 = ps3p.tile([128, NP, W], f32)
                    nc.tensor.matmul(ps3[32 * g:32 * g + 32], wl_sb[:], ds[:],
                                     start=True, stop=True, tile_position=(0, 32 * g))
                    if g == 3:
                        lg = lgp.tile([128, NP, W], f32)
                        nc.scalar.activation(out=lg[:], in_=ps3[:], func=AF.Ln,
                                             bias=bl_sb[:, 0:1], scale=1.0)
                        nc.vector.scalar_tensor_tensor(out=acc[:], in0=lg[:],
                                                       scalar=0.0, in1=acc[:],
                                                       op0=ALU.add, op1=ALU.add)
                for img in range(NP):
                    for c in range(C):
                        src = AP(tensor=oaa.tensor,
                                 offset=oaa.offset + (c * 8) * OAI + img * NB * W,
                                 ap=[[OAI, 8], [W, NB], [1, W]])
                        dst = AP(tensor=ys.ap().tensor,
                                 offset=(pair * NP + img) * C * H * W + c * H * W,
                                 ap=[[W, 8], [8 * W, NB], [1, W]])
                        nc.sync.dma_start(out=dst, in_=src)
            # ---- final: per-partition sum of acc -> lds
            lgf = lgp.tile([128, NP, W], f32)
            nc.scalar.activation(out=lgf[:], in_=acc[:], func=AF.Identity,
                                 bias=0.0, scale=1.0, accum_out=ld_sb[:])
            nc.sync.dma_start(out=lds.ap().unsqueeze(1), in_=ld_sb[:])

    nc.compile()
    return nc


def kernel(x, log_det, w1, b1, centers, w2, b2, res):
    x = np.ascontiguousarray(np.asarray(x, np.float32))
    log_det = np.asarray(log_det, np.float32)
    consts = _prep_consts(np.asarray(w1, np.float32), np.asarray(b1, np.float32),
                          np.asarray(centers, np.float32), np.asarray(w2, np.float32),
                          np.asarray(b2, np.float32), np.asarray(res, np.float32))

    if "nc" not in _cache:
        _cache["nc"] = _build_nc()
    nc = _cache["nc"]

    from concourse.bass_utils import run_bass_kernel_spmd
    in_maps = []
    for cid in range(N_CORES):
        m = dict(consts)
        m["xs"] = x[cid * NI:(cid + 1) * NI]
        in_maps.append(m)
    res_ = run_bass_kernel_spmd(nc, in_maps, core_ids=list(range(N_CORES)))

    out = np.concatenate([r["ys"] for r in res_.results], axis=0)
    ld_total = np.sum([np.sum(r["lds"], dtype=np.float64) for r in res_.results])
    log_det_out = (log_det.astype(np.float64) + ld_total).astype(np.float32)
    log_det_out = np.asarray(log_det_out, np.float32).reshape(np.shape(log_det))
    return out, log_det_out


# revision 17
# speedup vs baseline: 2.4102x; 1.3954x over previous
"""Trainium2 Bass kernel for nn_BasicBlockA (masked-conv flow block).

Computation: two 3x3 convs with tiny channel counts (3->12, 12->3),
leaky-relu between, residual add, plus a log-det reduction.

Mapping: "banded stationary" matmuls.  A block of 8 output rows is computed
per step; the PE contraction dim holds (channel, band-row) pairs and the
3x3 conv's dy-taps live inside a banded stationary matrix, while the dx-taps
become 3 PSUM-accumulating matmuls whose moving operands are x-shifted views
of the same SBUF tile.  conv1 emits lat1 in exactly the (channel, band-row)
partition layout conv2 consumes, so no on-chip relayout is needed.

Packing: conv1 runs two blocks concurrently on PE row-groups 0 and 2
(stationaries duplicated at partition 64); conv2 and the log-det diag-sum
run four blocks concurrently on PE column-groups (tile_position), sharing
one PSUM bank, so the residual/output fixup and the Ln pass process 128
partitions at once.  Residual x and outputs live in column-group-shaped
[128, img, group, W] SBUF tiles moved by a few large DMAs per image pair.

Data parallel over batch: 4 images per core x 8 cores; images processed in
pairs side-by-side in the free dim (2 x 258 incl. zero-pad columns -> N=512).
"""
import os
import sys

sys.path.insert(0, "/opt/trn_rl_repo")

import numpy as np

L, C, KK, MID = 4, 3, 3, 1
SLOPE = 0.1
H = W = 256
N_CORES = 8
NI = 4            # images per core
NP = 2            # images per pair (batched in matmul free dim)
NB = H // 8       # 8-output-row blocks per pair
NG = NB // 4      # 4-block groups per pair
FP = W + 2        # padded row length

CONV_DTYPE = os.environ.get("BASS_CONV_DTYPE", "bf16")   # "bf16" | "fp32"

_cache = {}


def _masks():
    m0 = np.ones((C, C, KK, KK), np.float32)
    m1 = np.zeros((C, C, KK, KK), np.float32)
    m = np.ones((C, C, KK, KK), np.float32)
    for i in range(C):
        m0[i, i, MID, MID] = 0.0
        m1[i, i, MID, MID] = 1.0
        m[i, :, MID + 1:, :] = 0.0
        m[i, :i + 1, MID, MID + 1:] = 0.0
        m[i, i + 1:, MID, MID:] = 0.0
    return m0, m1, m


def _prep_consts(w1, b1, centers, w2, b2, res):
    """Host-side: fold masks/weights into banded stationaries + bias vectors."""
    m0, m1, m = _masks()
    W1f = ((w1 * m0 + centers * m1) * m).reshape(L * C, C, KK, KK)
    w2r = w2.reshape(L, L, C, C, KK, KK)
    Wsum = (w2r.sum(0) * m0 + L * (centers * m1)) * m
    Wcat = Wsum.transpose(1, 0, 2, 3, 4).reshape(C, L * C, KK, KK)
    bsum = b2.sum(0)
    cdsq = centers[:, np.arange(C), np.arange(C), MID, MID] ** 2   # [L, C]
    S_c = cdsq.sum(0)
    rm = float(res[0] * (res[0] > 0))
    b1f = b1.reshape(L * C)

    # conv1 stationaries [variant(0=mid,1=top,2=bot)][dx][36=(yl,ci)][120=(co,yo)]
    w1s = np.zeros((3, 3, 36, 120), np.float32)
    yl = np.arange(12)[:, None]
    yo = np.arange(10)[None, :]
    band = (yl - yo >= 0) & (yl - yo <= 2)
    for var in range(3):
        for dx in range(3):
            for ci in range(C):
                for co in range(L * C):
                    blk = np.where(band, W1f[co, ci, np.clip(yl - yo, 0, 2), dx], 0.0)
                    if var == 1:
                        blk = blk.copy(); blk[:, 0] = 0.0
                    if var == 2:
                        blk = blk.copy(); blk[:, 9] = 0.0
                    w1s[var, dx, ci::3, co * 10:co * 10 + 10] = blk
    # conv2 stationaries [dx][120=(ci2,yl2)][32=(c,yo2)+8 zero pad cols]
    w2s = np.zeros((3, 120, 32), np.float32)
    yl2 = np.arange(10)[:, None]
    yo2 = np.arange(8)[None, :]
    band2 = (yl2 - yo2 >= 0) & (yl2 - yo2 <= 2)
    for dx in range(3):
        for ci2 in range(L * C):
            for c in range(C):
                blk = np.where(band2, Wcat[c, ci2, np.clip(yl2 - yo2, 0, 2), dx], 0.0)
                w2s[dx, ci2 * 10:ci2 * 10 + 10, c * 8:c * 8 + 8] = blk
    # log-det sum stationary [120=((l,c),yo)][32=(c,yo)+2pad]
    wls = np.zeros((120, 32), np.float32)
    for l in range(L):
        for c in range(C):
            for yob in range(1, 9):
                wls[(l * 3 + c) * 10 + yob, c * 10 + yob] = 0.9 * cdsq[l, c]
    # conv1 bias variants [3][120]
    b1v = np.tile(np.repeat(b1f, 10).astype(np.float32), (3, 1))
    b1v[1, 0::10] = 0.0
    b1v[2, 9::10] = 0.0
    # log bias [128]: rm + 0.1*S_c interior, 1.0 elsewhere (-> ln 0)
    bl = np.ones(128, np.float32)
    for g in range(4):
        for c in range(C):
            bl[32 * g + c * 10 + 1:32 * g + c * 10 + 9] = rm + 0.1 * S_c[c]
    # residual affine vectors in column-group layout p = 32j + c*8 + yo2
    rmv = np.full(128, rm, np.float32)
    b2v = np.zeros(128, np.float32)
    for j in range(4):
        for c in range(C):
            b2v[32 * j + c * 8:32 * j + c * 8 + 8] = bsum[c] / 16.0
    # device tiles are partition-major: [36,(var,dx),120] and [120,(dx),32]
    w1s = np.ascontiguousarray(w1s.transpose(2, 0, 1, 3))
    w2s = np.ascontiguousarray(w2s.transpose(1, 0, 2))
    if CONV_DTYPE == "bf16":
        import ml_dtypes
        w1s = w1s.astype(ml_dtypes.bfloat16)
        w2s = w2s.astype(ml_dtypes.bfloat16)
    return {
        "w1s": w1s, "w2s": w2s, "rmv": rmv, "wls": wls,
        "b1s": b1v, "bls": bl, "b2s": b2v,
    }


def _build_nc():
    import concourse.bacc as bacc
    import concourse.bass as bass
    import concourse.tile as tile
    from concourse import mybir

    f32 = mybir.dt.float32
    ct = mybir.dt.bfloat16 if CONV_DTYPE == "bf16" else f32
    AF = mybir.ActivationFunctionType
    ALU = mybir.AluOpType
    AP = bass.AP

    nc = bacc.Bacc("TRN2", target_bir_lowering=False, debug=False,
                   num_devices=N_CORES)
    xs = nc.dram_tensor("xs", [NI // NP, H, C, NP, W], f32, kind="ExternalInput")
    w1s = nc.dram_tensor("w1s", [36, 3, 3, 120], ct, kind="ExternalInput")
    w2s = nc.dram_tensor("w2s", [120, 3, 32], ct, kind="ExternalInput")
    rmv = nc.dram_tensor("rmv", [128], f32, kind="ExternalInput")
    wls = nc.dram_tensor("wls", [120, 32], f32, kind="ExternalInput")
    b1s = nc.dram_tensor("b1s", [3, 120], f32, kind="ExternalInput")
    bls = nc.dram_tensor("bls", [128], f32, kind="ExternalInput")
    b2s = nc.dram_tensor("b2s", [128], f32, kind="ExternalInput")
    ys = nc.dram_tensor("ys", [NI // NP, H, C, NP, W], f32, kind="ExternalOutput")
    lds = nc.dram_tensor("lds", [128], f32, kind="ExternalOutput")

    with tile.TileContext(nc) as tc:
        with (
            tc.tile_pool(name="singles", bufs=1) as singles,
            tc.tile_pool(name="xfp", bufs=1) as xfp,
            tc.tile_pool(name="dsp", bufs=6) as dsp,
            tc.tile_pool(name="outp", bufs=1) as outp,
            tc.tile_pool(name="lgp", bufs=2) as lgp,
            tc.tile_pool(name="ps1p", bufs=2, space="PSUM") as ps1p,
            tc.tile_pool(name="ps2p", bufs=2, space="PSUM") as ps2p,
            tc.tile_pool(name="ps3p", bufs=2, space="PSUM") as ps3p,
        ):
            # conv1 stationaries live at partitions 0..35 AND 64..99 so two
            # blocks can run on separate PE row-groups concurrently
            w1_sb = singles.tile([100, 3, 3, 120], ct)
            nc.sync.dma_start(out=w1_sb[0:36], in_=w1s.ap())
            nc.sync.dma_start(out=w1_sb[64:100], in_=w1s.ap())
            w2_sb = singles.tile([120, 3, 32], ct)
            nc.sync.dma_start(out=w2_sb[:], in_=w2s.ap())
            rm_sb = singles.tile([128, 1], f32)
            nc.sync.dma_start(out=rm_sb[:], in_=rmv.ap().unsqueeze(1))
            wl_sb = singles.tile([120, 32], f32)
            nc.sync.dma_start(out=wl_sb[:], in_=wls.ap())
            b1_sb = singles.tile([120, 3], f32)
            for var in range(3):
                nc.sync.dma_start(out=b1_sb[:, var:var + 1],
                                  in_=b1s.ap()[var:var + 1, :].transpose([1, 0]))
            bl_sb = singles.tile([128, 1], f32)
            nc.sync.dma_start(out=bl_sb[:], in_=bls.ap().unsqueeze(1))
            b2_sb = singles.tile([128, 1], f32)
            nc.sync.dma_start(out=b2_sb[:], in_=b2s.ap().unsqueeze(1))
            acc = singles.tile([128, NP, W], f32)
            nc.vector.memset(acc[:], 0.0)
            ld_sb = singles.tile([128, 1], f32)
            # persistent rotated buffers for the zero-padded tiles: pad columns
            # are zeroed once and never touched again
            xt_bufs = [singles.tile([100, NP, FP], ct, name=f"xtb{i}",
                                    tag=f"xtb{i}") for i in range(3)]
            lat_bufs = [singles.tile([120, NP, FP], ct, name=f"latb{i}",
                                     tag=f"latb{i}") for i in range(6)]
            for t in xt_bufs + lat_bufs:
                nc.vector.memset(t[:, :, 0:1], 0.0)
                nc.vector.memset(t[:, :, FP - 1:FP], 0.0)

            for pair in range(NI // NP):
                # residual input, col-group layout [128=(j,c,yo2), img, go, x]
                xf_all = xfp.tile([128, NP, NG, W], f32)
                xfa = xf_all[:]
                XFI = xfa.ap[0][0]
                nc.vector.memset(xf_all[:], 0.0)
                RS = C * NP * W
                for img in range(NP):
                    for c in range(C):
                        for j in range(4):
                            dst = AP(tensor=xfa.tensor,
                                     offset=xfa.offset + (32 * j + 8 * c) * XFI
                                     + img * NG * W,
                                     ap=[[XFI, 8], [W, NG], [1, W]])
                            src = AP(tensor=xs.ap().tensor,
                                     offset=pair * H * RS + (8 * j) * RS
                                     + c * NP * W + img * W,
                                     ap=[[RS, 8], [32 * RS, NG], [1, W]])
                            nc.sync.dma_start(out=dst, in_=src)
                nc.vector.tensor_scalar(out=xf_all[:], in0=xf_all[:],
                                        scalar1=rm_sb[:, 0:1], scalar2=b2_sb[:, 0:1],
                                        op0=ALU.mult, op1=ALU.add)
                out_all = outp.tile([128, NP, NG, W], f32)
                oaa = out_all[:]
                OAI = oaa.ap[0][0]
                for go in range(NG):
                    lats, dss = [], []
                    for ss in range(2):
                        bb = 4 * go + 2 * ss
                        tglob = (pair * NB + bb) // 2
                        xt = xt_bufs[tglob % 3]
                        xta = xt[:]
                        XPI = xta.ap[0][0]
                        if bb + 1 == NB - 1:
                            nc.vector.memset(xt[:, :, 1:W + 1], 0.0)
                        if bb == 0:
                            nc.vector.memset(xt[0:6, :, 1:W + 1], 0.0)
                        # ---- load both blocks' x bands (rows 8b-2..8b+9)
                        for s in range(2):
                            b = bb + s
                            yl0, nyl = 0, 12
                            if b == 0:
                                yl0, nyl = 2, 10
                            if b == NB - 1:
                                nyl = 10
                            r0 = 8 * b - 2 + yl0
                            dst = AP(tensor=xta.tensor,
                                     offset=xta.offset
                                     + (64 * s + yl0 * 3) * XPI + 1,
                                     ap=[[XPI, nyl * 3], [FP, NP], [1, W]])
                            src = AP(tensor=xs.ap().tensor,
                                     offset=pair * H * RS + r0 * RS,
                                     ap=[[1, nyl * RS]])
                            nc.gpsimd.dma_start(out=dst, in_=src)
                        # ---- conv1 for both blocks on PE row-groups 0 / 2
                        pss = [ps1p.tile([120, NP, W], f32, name=f"ps1_{go}_{ss}_{s2}",
                                         tag=f"ps1{s2}") for s2 in range(2)]
                        for dx in range(3):
                            for s in range(2):
                                var = (1 if bb + s == 0 else
                                       (2 if bb + s == NB - 1 else 0))
                                rhs = AP(tensor=xta.tensor,
                                         offset=xta.offset + 64 * s * XPI + dx,
                                         ap=[[XPI, 36], [FP, NP], [1, W]])
                                nc.tensor.matmul(pss[s][:],
                                                 w1_sb[64 * s:64 * s + 36, var, dx, :],
                                                 rhs, start=(dx == 0), stop=(dx == 2),
                                                 tile_position=(64 * s, 0))
                        for s in range(2):
                            b = bb + s
                            var = 1 if b == 0 else (2 if b == NB - 1 else 0)
                            tg2 = pair * NB + b
                            lat = lat_bufs[tg2 % 6]
                            nc.scalar.activation(out=lat[:, :, 1:W + 1], in_=pss[s][:],
                                                 func=AF.Prelu,
                                                 bias=b1_sb[:, var:var + 1],
                                                 scale=1.0, alpha=SLOPE)
                            ds = dsp.tile([120, NP, W], f32)
                            nc.vector.tensor_scalar(out=ds[:],
                                                    in0=lat[:, :, 1:W + 1],
                                                    scalar1=0.0, scalar2=None,
                                                    op0=ALU.is_gt)
                            lats.append(lat)
                            dss.append(ds)
                    # ---- conv2: 4 blocks packed on PE column groups, one bank
                    ps2g = ps2p.tile([128, NP, W], f32)
                    for dx in range(3):
                        for j in range(4):
                            lata = lats[j][:]
                            LPI = lata.ap[0][0]
                            rhs = AP(tensor=lata.tensor, offset=lata.offset + dx,
                                     ap=[[LPI, 120], [FP, NP], [1, W]])
                            nc.tensor.matmul(ps2g[32 * j:32 * j + 32],
                                             w2_sb[:, dx, :], rhs,
                                             start=(dx == 0), stop=(dx == 2),
                                             tile_position=(0, 32 * j))
                    # ---- out = psum2/16 + (bsum/16 + rm*x) -> out_all[:, :, go, :]
                    xfs_sl = AP(tensor=xfa.tensor, offset=xfa.offset + go * W,
                                ap=[[XFI, 128], [NG * W, NP], [1, W]])
                    out_sl = AP(tensor=oaa.tensor, offset=oaa.offset + go * W,
                                ap=[[OAI, 128], [NG * W, NP], [1, W]])
                    nc.vector.scalar_tensor_tensor(out=out_sl, in0=ps2g[:],
                                                   scalar=1.0 / 16.0, in1=xfs_sl,
                                                   op0=ALU.mult, op1=ALU.add)
                    # ---- log-det: 4 packed diag-sum matmuls, Ln, accumulate
                    ps3 = ps3p.tile([128, NP, W], f32)
                    for j in range(4):
                        nc.tensor.matmul(ps3[32 * j:32 * j + 32], wl_sb[:], dss[j],
                                         start=True, stop=True,
                                         tile_position=(0, 32 * j))
                    lg = lgp.tile([128, NP, W], f32)
                    nc.scalar.activation(out=lg[:], in_=ps3[:], func=AF.Ln,
                                         bias=bl_sb[:, 0:1], scale=1.0)
                    nc.vector.scalar_tensor_tensor(out=acc[:], in0=lg[:],
                                                   scalar=0.0, in1=acc[:],
                                                   op0=ALU.add, op1=ALU.add)
                # ---- store the pair's outputs: 24 large DMAs
                for img in range(NP):
                    for c in range(C):
                        for j in range(4):
                            src = AP(tensor=oaa.tensor,
                                     offset=oaa.offset + (32 * j + 8 * c) * OAI
                                     + img * NG * W,
                                     ap=[[OAI, 8], [W, NG], [1, W]])
                            dst = AP(tensor=ys.ap().tensor,
                                     offset=pair * H * RS + (8 * j) * RS
                                     + c * NP * W + img * W,
                                     ap=[[RS, 8], [32 * RS, NG], [1, W]])
                            nc.sync.dma_start(out=dst, in_=src)
            # ---- final: per-partition sum of acc -> lds
            lgf = lgp.tile([128, NP, W], f32)
            nc.scalar.activation(out=lgf[:], in_=acc[:], func=AF.Identity,
                                 bias=0.0, scale=1.0, accum_out=ld_sb[:])
            nc.sync.dma_start(out=lds.ap().unsqueeze(1), in_=ld_sb[:])

    nc.compile()
    return nc


def kernel(x, log_det, w1, b1, centers, w2, b2, res):
    x = np.ascontiguousarray(np.asarray(x, np.float32))
    log_det = np.asarray(log_det, np.float32)
    consts = _prep_consts(np.asarray(w1, np.float32), np.asarray(b1, np.float32),
                          np.asarray(centers, np.float32), np.asarray(w2, np.float32),
                          np.asarray(b2, np.float32), np.asarray(res, np.float32))

    if "nc" not in _cache:
        _cache["nc"] = _build_nc()
    nc = _cache["nc"]

    from concourse.bass_utils import run_bass_kernel_spmd
    in_maps = []
    for cid in range(N_CORES):
        m = dict(consts)
        shard = x[cid * NI:(cid + 1) * NI]
        m["xs"] = np.ascontiguousarray(
            shard.reshape(NI // NP, NP, C, H, W).transpose(0, 3, 2, 1, 4))
        in_maps.append(m)
    res_ = run_bass_kernel_spmd(nc, in_maps, core_ids=list(range(N_CORES)))

    out = np.concatenate(
        [r["ys"].transpose(0, 3, 2, 1, 4).reshape(NI, C, H, W)
         for r in res_.results], axis=0)
    ld_total = np.sum([np.sum(r["lds"], dtype=np.float64) for r in res_.results])
    log_det_out = (log_det.astype(np.float64) + ld_total).astype(np.float32)
    log_det_out = np.asarray(log_det_out, np.float32).reshape(np.shape(log_det))
    return out, log_det_out
